# revision 28
# baseline (speedup 1.0000x reference)
"""Causal self-attention (B=2, S=2048, E=1024, H=16, D=64) on 8 trn2 NeuronCores.

Sharding: tensor-parallel over heads — 2 heads per core. Each core computes
qkv^T = (W_qkv_c)^T x^T for its 3*128 qkv dims, runs causal attention for its
2 heads, and multiplies by its 128-row slice of W_proj, producing a partial
[4096, 1024] output (bf16). The host sums the 8 partials and adds b_proj.

Engine split (per core):
  PE    : qkv GEMMs; row-packed Q@K^T scores (2 heads concurrently via
          tile_position, causally column-trimmed); causal -1e30 mask added
          into the score PSUM by an identity-weight matmul (accumulation
          group {scores, ident@mask}); AV with a ones-column so the softmax
          denominator accumulates in PSUM row 64; proj.
  ACT   : exp(0.125*s) only.
  DVE   : qkv bias-add evacuation, proj PSUM evacuation (fp32->bf16),
          denominator copy + fast reciprocal, final normalize-mul into yT.
  GpSimd: V^T copies into the ones-augmented V tile + reciprocal broadcast.

Causal trimming: for diagonal k-tile o of a 512-wide q-chunk only columns
[128*o, 512) are computed/masked/exp'd/accumulated.

PSUM (8 banks): scores pg [128,2,512] x2 bufs = 4; AV py [65,512] x2 heads
= 2; qkv/proj/vT work tile [128,512] x2 bufs = 2.
"""

import os
import sys

if "/opt/trn_rl_repo" not in sys.path:
    sys.path.insert(0, "/opt/trn_rl_repo")

import numpy as np

import concourse.bass as bass  # noqa: F401
import concourse.mybir as mybir
import concourse.tile as tile
from concourse import bacc
from concourse.bass_utils import run_bass_kernel_spmd
from concourse.masks import make_identity

B, S, E, H, D = 2, 2048, 1024, 16, 64
NCORES = 8
BS = B * S                   # 4096
CH = 512                     # column chunk of x^T / qkv^T / q-chunk
NCH = BS // CH               # 8 chunks
KT = S // 128                # 16 k-tiles per batch
f32 = mybir.dt.float32
bf16 = mybir.dt.bfloat16
DT = bf16
MASK_VAL = -1e30
VT_MODE = os.environ.get("VT", "dma")  # "dma" (XBAR transpose via temp) | "pe"


def build_nc():
    nc = bacc.Bacc(None, target_bir_lowering=False)
    xT = nc.dram_tensor("xT", [E, BS], DT, kind="ExternalInput")
    wqkv = nc.dram_tensor("wqkv", [E, 3 * 128], DT, kind="ExternalInput")
    bqkv = nc.dram_tensor("bqkv", [128, 3], f32, kind="ExternalInput")
    wproj = nc.dram_tensor("wproj", [128, E], DT, kind="ExternalInput")
    maskb = nc.dram_tensor("maskb", [128, 4, CH], DT, kind="ExternalInput")
    out = nc.dram_tensor("out", [BS, E], DT, kind="ExternalOutput")

    with tile.TileContext(nc) as tc:
        with (
            tc.tile_pool(name="singles", bufs=1) as singles,
            tc.tile_pool(name="xpool", bufs=16) as xpool,
            tc.tile_pool(name="ppool", bufs=6) as ppool,
            tc.tile_pool(name="npool", bufs=2) as npool,
            tc.tile_pool(name="opool", bufs=4) as opool,
            tc.tile_pool(name="vtpool", bufs=32) as vtpool,
            tc.tile_pool(name="ps_s", bufs=2, space="PSUM") as ps_s,
            tc.tile_pool(name="ps_y", bufs=1, space="PSUM") as ps_y,
            tc.tile_pool(name="ps_w", bufs=2, space="PSUM") as ps_w,
        ):
            # ---- persistent tiles ----
            wqkv_sb = singles.tile([128, 8, 384], DT, tag="wqkv")
            nc.sync.dma_start(
                out=wqkv_sb, in_=wqkv.rearrange("(ko ki) m -> ki ko m", ki=128)
            )
            bqkv_sb = singles.tile([128, 3], f32, tag="bqkv")
            nc.sync.dma_start(out=bqkv_sb, in_=bqkv[:, :])
            wproj_sb = singles.tile([128, E], DT, tag="wproj")
            nc.sync.dma_start(out=wproj_sb, in_=wproj[:, :])
            maskb_sb = singles.tile([128, 4, CH], DT, tag="maskb")
            nc.sync.dma_start(out=maskb_sb, in_=maskb[:, :, :])
            ident = singles.tile([128, 128], DT, tag="ident")
            make_identity(nc, ident[:])

            qkvT = [
                singles.tile([128, 3, CH], DT, tag=f"qkvT{n}", name=f"qkvT{n}")
                for n in range(NCH)
            ]
            # V_aug per batch: [128, kt, 130]; cols 0:64 head0 V^T, col 64
            # ones, 65:129 head1 V^T, col 129 ones.
            vaug = [
                singles.tile([128, KT, 130], DT, tag=f"vaug{b}", name=f"vaug{b}")
                for b in range(B)
            ]
            ones_sb = singles.tile([128, KT], f32, tag="ones")
            nc.vector.memset(ones_sb[:], 1.0)
            for b in range(B):
                nc.vector.tensor_copy(out=vaug[b][:, :, 64:65], in_=ones_sb[:])
                nc.vector.tensor_copy(out=vaug[b][:, :, 129:130], in_=ones_sb[:])
            yT = [
                singles.tile([128, CH], DT, tag=f"yT{n}", name=f"yT{n}")
                for n in range(NCH)
            ]

            # exp table preload: dummy activation so the ~2.7us ACT_TABLE_LOAD
            # happens during the initial DMA prefetch, not at first real exp
            dumm = singles.tile([128, 1], f32, tag="dumm")
            nc.scalar.activation(
                out=dumm[:], in_=ones_sb[:, 0:1],
                func=mybir.ActivationFunctionType.Exp,
            )

            # HAM warmup: the PE otherwise idles ~18us waiting for the x
            # prefetch and then runs the whole qkv phase at K=4/8 half clock
            # (~3.4us of sustained busy is needed to unthrottle). Burn the DMA
            # wait with throwaway ident@ident matmuls so real work starts warm.
            warm_ps = ps_w.tile([128, 128], f32, tag="w", name="warm")
            for i in range(200):
                nc.tensor.matmul(
                    warm_ps[:, 0:128], ident[:], ident[:], start=True, stop=True
                )

            # all x loads issued up-front as 16 big [128,2048] DMAs (4 chunks
            # per transfer): the sync DMA ring is FIFO per engine, so dependent
            # DMAs (out stores) must queue BEHIND every pure-prefetch load or
            # they head-of-line block them; fewer+bigger transfers stream
            # closer to peak HBM rate.
            xgrp = {}
            for g in range(2):
                for k in range(8):
                    t = xpool.tile([128, 4, CH], DT, tag="xt", name=f"x{g}_{k}")
                    nc.sync.dma_start(
                        out=t,
                        in_=xT[k * 128:(k + 1) * 128, g * 2048:(g + 1) * 2048]
                        .rearrange("p (c q) -> p c q", c=4),
                    )
                    xgrp[(g, k)] = t
            xtiles = {
                (n, k): xgrp[(n // 4, k)][:, n % 4, :]
                for n in range(NCH) for k in range(8)
            }

            def qkv_chunk(n):
                for m in range(3):
                    ww = ps_w.tile([128, CH], f32, tag="w", name=f"qkv{n}_{m}")
                    for k in range(8):
                        nc.tensor.matmul(
                            ww[:],
                            wqkv_sb[:, k, m * 128:(m + 1) * 128],
                            xtiles[(n, k)][:],
                            start=(k == 0),
                            stop=(k == 7),
                        )
                    nc.vector.tensor_scalar_add(
                        out=qkvT[n][:, m, :], in0=ww[:],
                        scalar1=bqkv_sb[:, m:m + 1],
                    )

            vt_pending = []

            def vt_chunk(b, n):
                # V^T k-tiles for chunk n: XBAR transposes on the scalar-hosted
                # HWDGE ring (sync ring stays pure loads/stores, PE stays
                # free). The vaug copies are DEFERRED: they go on gpsimd whose
                # stream must hold nothing else until all copies are emitted —
                # any engine stream is strict FIFO, so a copy waiting on its
                # transpose would head-of-line block whatever comes after it.
                for j in range(4):
                    kt = 4 * (n % 4) + j
                    tv = vtpool.tile([128, 128], DT, tag="tv")
                    nc.scalar.dma_start_transpose(
                        tv[:, :], qkvT[n][:, 2, 128 * j:128 * (j + 1)]
                    )
                    vt_pending.append((b, kt, tv))

            def vt_flush():
                for b, kt, tv in vt_pending:
                    nc.gpsimd.tensor_copy(out=vaug[b][:, kt, 0:64], in_=tv[:, 0:64])
                    nc.gpsimd.tensor_copy(
                        out=vaug[b][:, kt, 65:129], in_=tv[:, 64:128]
                    )
                vt_pending.clear()

            def attention_qc(b, qc):
                nq = b * 4 + qc
                ktmax = 4 * (qc + 1)
                py = [
                    ps_y.tile([65, CH], f32, tag=f"y{h}", name=f"py{b}_{qc}_{h}")
                    for h in range(2)
                ]

                def emit_av(kt, pt_sb, off):
                    for h in range(2):
                        nc.tensor.matmul(
                            py[h][:, off:CH],
                            vaug[b][:, kt, h * 65:h * 65 + 65],
                            pt_sb[:, h, off:CH],
                            start=(kt == 0),
                            stop=(kt == ktmax - 1),
                            skip_group_check=True,
                        )

                pending = []  # software pipeline: AV(t) emitted after exp(t+2)
                for kt in range(ktmax):
                    o = kt - 4 * qc if kt >= 4 * qc else None  # diagonal index
                    off = 128 * o if o is not None else 0
                    nk = b * 4 + kt // 4
                    offk = (kt % 4) * 128
                    pg = ps_s.tile([128, 2, CH], f32, tag="s")
                    for h in range(2):
                        hb = h * 64
                        nc.tensor.matmul(
                            pg[:, h, off:CH],
                            qkvT[nk][hb:hb + 64, 1, offk:offk + 128],
                            qkvT[nq][hb:hb + 64, 0, off:CH],
                            start=True,
                            stop=(o is None),
                            tile_position=(hb, 0),
                            skip_group_check=True,
                        )
                    if o is not None:
                        # add -1e30 causal mask into the score PSUM: += I^T @ M
                        for h in range(2):
                            nc.tensor.matmul(
                                pg[:, h, off:CH],
                                ident[:],
                                maskb_sb[:, o, off:CH],
                                start=False,
                                stop=True,
                                skip_group_check=True,
                            )
                    pt_sb = ppool.tile([128, 2, CH], DT, tag="pT")
                    nc.scalar.activation(
                        out=pt_sb[:, :, off:CH],
                        in_=pg[:, :, off:CH],
                        func=mybir.ActivationFunctionType.Exp,
                        scale=0.125,
                    )
                    pending.append((kt, pt_sb, off))
                    if len(pending) > 2:
                        emit_av(*pending.pop(0))
                for p in pending:
                    emit_av(*p)
                # normalize both heads: rec = 1/den; yT = py * rec
                den2 = npool.tile([1, 2, CH], f32, tag="den")
                for h in range(2):
                    nc.vector.tensor_copy(out=den2[:, h, :], in_=py[h][64:65, :])
                rec2 = npool.tile([1, 2, CH], f32, tag="rec")
                nc.vector.reciprocal_approx_fast(out=rec2[:], in_=den2[:])
                bc = npool.tile([64, 2, CH], f32, tag="bc")
                nc.gpsimd.partition_broadcast(out_ap=bc[:], in_ap=rec2[:])
                for h in range(2):
                    nc.vector.tensor_mul(
                        out=yT[nq][h * 64:h * 64 + 64, :],
                        in0=py[h][0:64, :],
                        in1=bc[:, h, :],
                    )

            def proj(n):
                for st in range(4):
                    row0 = n * CH + st * 128
                    for j in range(2):
                        pp = ps_w.tile([128, CH], f32, tag="w", name=f"pj{n}_{st}_{j}")
                        nc.tensor.matmul(
                            pp[:],
                            yT[n][:, st * 128:(st + 1) * 128],
                            wproj_sb[:, j * CH:(j + 1) * CH],
                            start=True,
                            stop=True,
                        )
                        o_sb = opool.tile([128, CH], DT, tag="o")
                        nc.vector.tensor_copy(out=o_sb[:], in_=pp[:])
                        nc.sync.dma_start(
                            out=out[row0:row0 + 128, j * CH:(j + 1) * CH],
                            in_=o_sb[:],
                        )

            # engine streams execute IN ORDER; emission order is the PE
            # program. All qkv first (dense, keeps HAM warm), then the two
            # attention batches; proj(qc) delayed one chunk so its yT
            # dependency (normalize chain) has a chunk of slack.
            for n in range(NCH):
                qkv_chunk(n)
                vt_chunk(n // 4, n)
            vt_flush()
            prev = None
            for b in range(B):
                for qc in range(4):
                    attention_qc(b, qc)
                    if prev is not None:
                        proj(prev)
                    prev = b * 4 + qc
            proj(prev)

    nc.finalize()
    return nc


def make_core_inputs(x, W_attn, b_attn, W_proj):
    """Host-side sharding: slice per-core weights, transpose x, build masks."""
    np_dt = mybir.dt.np(DT)
    xT = np.ascontiguousarray(x.reshape(BS, E).T).astype(np_dt)  # [E, BS]

    # additive causal masks for the 4 diagonal 128-row k-tiles of a 512-wide
    # q-chunk: valid iff j >= 128*o + i  (j = within-chunk q, i = k-in-tile)
    i = np.arange(128)[:, None]
    j = np.arange(CH)[None, :]
    maskb = np.stack(
        [np.where(j >= 128 * o + i, 0.0, MASK_VAL) for o in range(4)], axis=1
    ).astype(np_dt)  # [128, 4, 512]
    maskb = np.ascontiguousarray(maskb)

    in_maps = []
    for c in range(NCORES):
        cols = slice(128 * c, 128 * (c + 1))
        wqkv = np.ascontiguousarray(
            np.concatenate(
                [W_attn[:, cols], W_attn[:, E:][:, cols], W_attn[:, 2 * E:][:, cols]],
                axis=1,
            )
        ).astype(np_dt)  # [E, 384]
        bq = np.stack(
            [b_attn[cols], b_attn[E:][cols], b_attn[2 * E:][cols]], axis=1
        ).astype(np.float32)  # [128, 3]
        wp = np.ascontiguousarray(W_proj[128 * c:128 * (c + 1), :]).astype(np_dt)
        in_maps.append(
            {"xT": xT, "wqkv": wqkv, "bqkv": bq, "wproj": wp, "maskb": maskb}
        )
    return in_maps


_NC_CACHE = None


def kernel_run(inputs, trace=False):
    """Run the bass kernel; returns (full_output, BassKernelResults)."""
    global _NC_CACHE
    x = np.asarray(inputs["x"], dtype=np.float32)
    W_attn = np.asarray(inputs["W_attn"], dtype=np.float32)
    b_attn = np.asarray(inputs["b_attn"], dtype=np.float32)
    W_proj = np.asarray(inputs["W_proj"], dtype=np.float32)
    b_proj = np.asarray(inputs["b_proj"], dtype=np.float32)

    if _NC_CACHE is None:
        _NC_CACHE = build_nc()
    nc = _NC_CACHE

    in_maps = make_core_inputs(x, W_attn, b_attn, W_proj)
    res = run_bass_kernel_spmd(
        nc, in_maps, core_ids=list(range(NCORES)), trace=trace
    )
    acc = np.zeros((BS, E), dtype=np.float64)
    for r in res.results:
        acc += np.asarray(r["out"], dtype=np.float64)
    y = (acc + b_proj).astype(np.float32).reshape(B, S, E)
    return y, res


def kernel(**inputs):
    y, _ = kernel_run(inputs, trace=False)
    return y


if __name__ == "__main__":
    rng = np.random.default_rng(0)
    scale = 1.0 / np.sqrt(E)
    inputs = {
        "x": rng.standard_normal((B, S, E), dtype=np.float32),
        "W_attn": rng.standard_normal((E, 3 * E), dtype=np.float32) * scale,
        "b_attn": rng.standard_normal((3 * E,), dtype=np.float32) * 0.02,
        "W_proj": rng.standard_normal((E, E), dtype=np.float32) * scale,
        "b_proj": rng.standard_normal((E,), dtype=np.float32) * 0.02,
    }
    y = kernel(**inputs)
    print("kernel output", y.shape, y.dtype, float(np.abs(y).mean()))
